# revision 2
# baseline (speedup 1.0000x reference)
"""GAU (gated attention unit) Bass kernel for Trainium2, 8 NeuronCores.

Sharding: 8 cores = 4 batches x 2 sequence halves. Each core receives its
batch's x with ROWS REORDERED so its own q half comes first; it computes
k/v for all 4096 rows and attention output for rows 0..2047 (its q half).
Row order of kv is irrelevant to attention (sum over j).

Per-core pipeline (v2):
  phase A (kv loop, 8 groups of 512 rows): load x once, LN stats on DVE
    with one batched ACT Sqrt per group, normalize on DVE (f32r out),
    PE-transpose (f32r identity), Z projection -> kt (and qt for groups
    0-3), v projection -> bf16 SBUF, gate projection (groups 0-3) -> bf16
    SBUF (no HBM spill). ACT only runs Silu/Sqrt; Relu/Silu/Sqrt table
    swaps minimized.
  phase C (4 i-blocks of 512 q rows): simT = kT.T @ qT f32r, A=relu^2 ->
    bf16 (ACT relu + DVE square), V = A.T @ v bf16 accumulating in PSUM,
    V *= gate (bf16), PE-transpose bf16, out = Vg @ Wo bf16, + bo, DMA.

Bias matmuls (ones-row trick) are only emitted when the folded bias is
nonzero (compile-time specialization keyed on host-side values).
"""
import sys

sys.path.insert(0, "/opt/trn_rl_repo")

import numpy as np

import concourse.bass as bass
import concourse.mybir as mybir
from concourse import bacc
from concourse.masks import make_identity
from concourse.tile import TileContext

F32 = mybir.dt.float32
F32R = mybir.dt.float32r
BF16 = mybir.dt.bfloat16
AF = mybir.ActivationFunctionType
OP = mybir.AluOpType

S = 4096          # full sequence (kv rows per core)
SH = 2048         # per-core q rows
D = 512           # model dim
HID = 1024        # v / gate width
H2 = 2048         # 2*HID
QK = 128
OUT = 8
NKV = S // 128    # 32 kv seq tiles
NQ = SH // 128    # 16 q seq tiles
NG = S // 512     # 8 groups of 4 tiles
NGQ = SH // 512   # 4 q groups
NCORES = 8
LN_EPS = 1e-5

_nc_cache = {}


def _build(nreps=1, has_bias=False):
    nc = bacc.Bacc()

    xkv = nc.dram_tensor("xkv", [S, D], F32, kind="ExternalInput")
    wh = nc.dram_tensor("wh", [D, H2], F32, kind="ExternalInput")
    wqk = nc.dram_tensor("wqk", [D, QK], F32, kind="ExternalInput")
    bqk = nc.dram_tensor("bqk", [QK, 1], F32, kind="ExternalInput")
    gam0 = nc.dram_tensor("gam0", [QK, 1], F32, kind="ExternalInput")
    bet0 = nc.dram_tensor("bet0", [QK, 1], F32, kind="ExternalInput")
    gam1 = nc.dram_tensor("gam1", [QK, 1], F32, kind="ExternalInput")
    bet1 = nc.dram_tensor("bet1", [QK, 1], F32, kind="ExternalInput")
    wo = nc.dram_tensor("wo", [128, 8, OUT], F32, kind="ExternalInput")
    bo = nc.dram_tensor("bo", [1, OUT], F32, kind="ExternalInput")
    bh = (
        nc.dram_tensor("bh", [1, H2], F32, kind="ExternalInput")
        if has_bias
        else None
    )
    out_d = nc.dram_tensor("out", [SH, OUT], F32, kind="ExternalOutput")

    with TileContext(nc) as tc:
        with (
            tc.tile_pool(name="persist", bufs=1) as pers,
            tc.tile_pool(name="vpool", bufs=1) as vpool,
        ):
            # ---- persistent constants ----
            ident_f32 = pers.tile([128, 128], F32, tag="identf")
            make_identity(nc, ident_f32)
            ident_fr = pers.tile([128, 128], F32R, tag="identr")
            nc.vector.tensor_copy(out=ident_fr, in_=ident_f32)
            ident_bf = pers.tile([128, 128], BF16, tag="identb")
            nc.vector.tensor_copy(out=ident_bf, in_=ident_f32)

            bqk_col = pers.tile([128, 1], F32, tag="bqk")
            nc.sync.dma_start(out=bqk_col, in_=bqk[:])
            gam0_c = pers.tile([128, 1], F32, tag="g0")
            nc.sync.dma_start(out=gam0_c, in_=gam0[:])
            bet0_c = pers.tile([128, 1], F32, tag="be0")
            nc.sync.dma_start(out=bet0_c, in_=bet0[:])
            gam1_c = pers.tile([128, 1], F32, tag="g1")
            nc.sync.dma_start(out=gam1_c, in_=gam1[:])
            bet1_c = pers.tile([128, 1], F32, tag="be1")
            nc.sync.dma_start(out=bet1_c, in_=bet1[:])
            wo_stage = pers.tile([128, 8, OUT], F32, tag="wos")
            nc.sync.dma_start(out=wo_stage, in_=wo[:])
            wo_sb = pers.tile([128, 8, OUT], BF16, tag="wo")
            nc.vector.tensor_copy(out=wo_sb, in_=wo_stage)
            bo_bc = pers.tile([128, OUT], F32, tag="bo")
            nc.sync.dma_start(out=bo_bc, in_=bo[:].to_broadcast([128, OUT]))

            ones_row = None
            if has_bias:
                ones_f32 = pers.tile([1, 128], F32, tag="ones32")
                nc.vector.memset(ones_f32, 1.0)
                ones_row = pers.tile([1, 128], F32R, tag="ones")
                nc.vector.tensor_copy(out=ones_row, in_=ones_f32)

            # ---- persistent activations ----
            v_sb = vpool.tile([128, NKV, HID], BF16, tag="v")
            gate_sb = vpool.tile([128, NQ, HID], BF16, tag="gate")
            kt_sb = pers.tile([128, S], F32R, tag="kt")
            qt_sb = pers.tile([128, SH], F32R, tag="qt")

            import contextlib

            rep_ctx = (
                tc.For_i(0, nreps, 1) if nreps > 1 else contextlib.nullcontext()
            )
            with rep_ctx:
                _emit_body(
                    nc, tc,
                    dict(xkv=xkv, wh=wh, wqk=wqk, bh=bh, out_d=out_d),
                    dict(ident_fr=ident_fr, ident_bf=ident_bf,
                         bqk_col=bqk_col, gam0_c=gam0_c, bet0_c=bet0_c,
                         gam1_c=gam1_c, bet1_c=bet1_c, wo_sb=wo_sb,
                         bo_bc=bo_bc, ones_row=ones_row,
                         v_sb=v_sb, gate_sb=gate_sb, kt_sb=kt_sb, qt_sb=qt_sb),
                    has_bias,
                )

    nc.compile()
    return nc


def _emit_body(nc, tc, drams, sbufs, has_bias):
    xkv, wh, wqk, bh, out_d = (
        drams[k] for k in ("xkv", "wh", "wqk", "bh", "out_d")
    )
    ident_fr = sbufs["ident_fr"]
    ident_bf = sbufs["ident_bf"]
    bqk_col = sbufs["bqk_col"]
    gam0_c, bet0_c = sbufs["gam0_c"], sbufs["bet0_c"]
    gam1_c, bet1_c = sbufs["gam1_c"], sbufs["bet1_c"]
    wo_sb, bo_bc = sbufs["wo_sb"], sbufs["bo_bc"]
    ones_row = sbufs["ones_row"]
    v_sb, gate_sb = sbufs["v_sb"], sbufs["gate_sb"]
    kt_sb, qt_sb = sbufs["kt_sb"], sbufs["qt_sb"]

    # ================= phase A: LN + projections =================
    with (
        tc.tile_pool(name="wp", bufs=1) as wp,
        tc.tile_pool(name="xp", bufs=4) as xp,
        tc.tile_pool(name="sp", bufs=12) as sp,
        tc.tile_pool(name="nTp", bufs=2) as nTp,
        tc.tile_pool(name="zp", bufs=2) as zp,
        tc.tile_pool(name="psTr", bufs=2, space="PSUM") as psTr,
        tc.tile_pool(name="psZ", bufs=2, space="PSUM") as psZ,
        tc.tile_pool(name="psP", bufs=2, space="PSUM") as psP,
    ):
        # stage weights: wh -> bf16, wqk -> f32r
        whr = wp.tile([128, 4, H2], BF16, tag="whr")
        for c in range(4):
            wh_stage = wp.tile([128, H2], F32, tag="whs")
            nc.sync.dma_start(out=wh_stage, in_=wh[c * 128 : (c + 1) * 128, :])
            nc.vector.tensor_copy(out=whr[:, c, :], in_=wh_stage)
        wqkr = wp.tile([128, 4, QK], F32R, tag="wqkr")
        for c in range(4):
            wqk_stage = wp.tile([128, QK], F32, tag="wqs")
            nc.sync.dma_start(out=wqk_stage, in_=wqk[c * 128 : (c + 1) * 128, :])
            nc.vector.tensor_copy(out=wqkr[:, c, :], in_=wqk_stage)
        bh_row = None
        if has_bias:
            bh_stage = wp.tile([1, H2], F32, tag="bhs")
            nc.sync.dma_start(out=bh_stage, in_=bh[:])
            bh_row = wp.tile([1, H2], F32R, tag="bhr")
            nc.vector.tensor_copy(out=bh_row, in_=bh_stage)

        for g in range(NG):
            is_q = g < NGQ
            # -- LN: stats for 4 tiles, one batched sqrt --
            xts = []
            mvs = []
            varg = sp.tile([128, 4], F32, tag="varg")
            for t in range(4):
                xt = xp.tile([128, D], F32, tag="xt")
                nc.sync.dma_start(
                    out=xt,
                    in_=xkv[(g * 4 + t) * 128 : (g * 4 + t + 1) * 128, :],
                )
                xts.append(xt)
                st = sp.tile([128, 6], F32, tag="bnst")
                nc.vector.bn_stats(out=st, in_=xt)
                mv = sp.tile([128, 2], F32, tag="bnmv")
                nc.vector.bn_aggr(out=mv, in_=st)
                mvs.append(mv)
                nc.vector.tensor_scalar(
                    varg[:, t : t + 1], mv[:, 1:2], LN_EPS, None, OP.add
                )
            sqg = sp.tile([128, 4], F32, tag="sqg")
            nc.scalar.activation(out=sqg, in_=varg, func=AF.Sqrt)
            rstdg = sp.tile([128, 4], F32, tag="rstdg")
            nc.vector.reciprocal(out=rstdg, in_=sqg)
            nmrg = sp.tile([128, 4], F32, tag="nmrg")
            for t in range(4):
                nc.vector.tensor_scalar(
                    nmrg[:, t : t + 1], mvs[t][:, 0:1],
                    rstdg[:, t : t + 1], -1.0, OP.mult, OP.mult,
                )
            # -- normalize (f32r) + transpose --
            nT = nTp.tile([128, 4, 512], F32R, tag="nT")
            for t in range(4):
                nsc = sp.tile([128, D], F32R, tag="nsc")
                nc.vector.tensor_scalar(
                    nsc, xts[t], rstdg[:, t : t + 1], nmrg[:, t : t + 1],
                    OP.mult, OP.add,
                )
                ptr = psTr.tile([128, 4, 128], F32R, tag="ptr")
                for c in range(4):
                    nc.tensor.transpose(
                        ptr[:, c, :], nsc[:, c * 128 : (c + 1) * 128], ident_fr
                    )
                nc.vector.tensor_copy(
                    out=nT[:, :, t * 128 : (t + 1) * 128], in_=ptr
                )
            # -- Z projection -> kt (and qt) --
            psz = psZ.tile([128, 512], F32, tag="psz")
            for c in range(4):
                nc.tensor.matmul(
                    psz, wqkr[:, c, :], nT[:, c, :],
                    start=(c == 0), stop=(c == 3),
                )
            zs = zp.tile([128, 512], F32, tag="zs")
            nc.scalar.activation(out=zs, in_=psz, func=AF.Silu, bias=bqk_col)
            nc.vector.tensor_scalar(
                kt_sb[:, g * 512 : (g + 1) * 512], zs,
                gam1_c, bet1_c, OP.mult, OP.add,
            )
            if is_q:
                nc.vector.tensor_scalar(
                    qt_sb[:, g * 512 : (g + 1) * 512], zs,
                    gam0_c, bet0_c, OP.mult, OP.add,
                )
            # -- v projection (and gate for q groups) --
            for t in range(4):
                s_idx = g * 4 + t
                psp = psP.tile([128, HID], F32, tag="psp")
                for nh in range(2):
                    for c in range(4):
                        nc.tensor.matmul(
                            psp[:, nh * 512 : (nh + 1) * 512],
                            nT[:, c, t * 128 : (t + 1) * 128],
                            whr[:, c, nh * 512 : (nh + 1) * 512],
                            start=(c == 0), stop=(c == 3 and not has_bias),
                        )
                    if has_bias:
                        nc.tensor.matmul(
                            psp[:, nh * 512 : (nh + 1) * 512],
                            ones_row,
                            bh_row[0:1, nh * 512 : (nh + 1) * 512],
                            start=False, stop=True,
                        )
                nc.scalar.activation(out=v_sb[:, s_idx, :], in_=psp, func=AF.Silu)
                if is_q:
                    psg = psP.tile([128, HID], F32, tag="psp")
                    for nh in range(2):
                        for c in range(4):
                            nc.tensor.matmul(
                                psg[:, nh * 512 : (nh + 1) * 512],
                                nT[:, c, t * 128 : (t + 1) * 128],
                                whr[:, c, HID + nh * 512 : HID + (nh + 1) * 512],
                                start=(c == 0), stop=(c == 3 and not has_bias),
                            )
                        if has_bias:
                            nc.tensor.matmul(
                                psg[:, nh * 512 : (nh + 1) * 512],
                                ones_row,
                                bh_row[0:1, HID + nh * 512 : HID + (nh + 1) * 512],
                                start=False, stop=True,
                            )
                    nc.scalar.activation(
                        out=gate_sb[:, s_idx, :], in_=psg, func=AF.Silu
                    )

    # ================= phase C: attention =================
    with (
        tc.tile_pool(name="atp", bufs=2) as atp,
        tc.tile_pool(name="rtp", bufs=3) as rtp,
        tc.tile_pool(name="vgp", bufs=2) as vgp,
        tc.tile_pool(name="vgtp", bufs=2) as vgtp,
        tc.tile_pool(name="osp", bufs=2) as osp,
        tc.tile_pool(name="psSim", bufs=2, space="PSUM") as psSim,
        tc.tile_pool(name="psV", bufs=2, space="PSUM") as psV,
        tc.tile_pool(name="psT", bufs=1, space="PSUM") as psT,
        tc.tile_pool(name="psO", bufs=1, space="PSUM") as psO,
    ):
        for ib in range(SH // 512):
            at_h = []
            for half in range(2):
                ath = atp.tile([128, 16, 512], BF16, tag="ath")
                at_h.append(ath)
                for j in range(16):
                    jt = half * 16 + j
                    pss = psSim.tile([128, 512], F32, tag="pss")
                    nc.tensor.matmul(
                        pss,
                        kt_sb[:, jt * 128 : (jt + 1) * 128],
                        qt_sb[:, ib * 512 : (ib + 1) * 512],
                        start=True, stop=True,
                    )
                    rt = rtp.tile([128, 512], F32, tag="rt")
                    nc.scalar.activation(out=rt, in_=pss, func=AF.Relu)
                    nc.vector.tensor_mul(out=ath[:, j, :], in0=rt, in1=rt)
            for t in range(4):
                i_idx = ib * 4 + t
                psv = psV.tile([128, HID], F32, tag="psv")
                for jt in range(NKV):
                    a_sl = at_h[jt // 16][:, jt % 16, t * 128 : (t + 1) * 128]
                    for nh in range(2):
                        nc.tensor.matmul(
                            psv[:, nh * 512 : (nh + 1) * 512],
                            a_sl,
                            v_sb[:, jt, nh * 512 : (nh + 1) * 512],
                            start=(jt == 0), stop=(jt == NKV - 1),
                        )
                vg = vgp.tile([128, HID], BF16, tag="vg")
                nc.vector.tensor_mul(out=vg, in0=psv, in1=gate_sb[:, i_idx, :])
                pst = psT.tile([128, 8, 128], BF16, tag="pst")
                for hc in range(8):
                    nc.tensor.transpose(
                        pst[:, hc, :], vg[:, hc * 128 : (hc + 1) * 128], ident_bf
                    )
                vgt = vgtp.tile([128, 8, 128], BF16, tag="vgt")
                nc.vector.tensor_copy(out=vgt, in_=pst)
                pso = psO.tile([128, OUT], F32, tag="pso")
                for hc in range(8):
                    nc.tensor.matmul(
                        pso, vgt[:, hc, :], wo_sb[:, hc, :],
                        start=(hc == 0), stop=(hc == 7),
                    )
                osb = osp.tile([128, OUT], F32, tag="osb")
                nc.vector.tensor_add(out=osb, in0=pso, in1=bo_bc)
                nc.sync.dma_start(
                    out=out_d[i_idx * 128 : (i_idx + 1) * 128, :], in_=osb
                )


def _get_nc(nreps=1, has_bias=False):
    key = (nreps, has_bias)
    if key not in _nc_cache:
        _nc_cache[key] = _build(nreps, has_bias)
    return _nc_cache[key]


def _prep_in_maps(inputs):
    return _prep(**inputs)[1]


def _prep(x, ln_g, ln_b, Wh, bh, Wqk, bqk, gamma, beta, Wo, bo):
    x = np.asarray(x, dtype=np.float32)
    f = lambda a: np.ascontiguousarray(np.asarray(a, dtype=np.float32))
    ln_g = np.asarray(ln_g, np.float64)
    ln_b = np.asarray(ln_b, np.float64)
    Whf = np.asarray(Wh, np.float64) * ln_g[:, None]
    bhf = np.asarray(bh, np.float64) + ln_b @ np.asarray(Wh, np.float64)
    Wqkf = np.asarray(Wqk, np.float64) * ln_g[:, None]
    bqkf = np.asarray(bqk, np.float64) + ln_b @ np.asarray(Wqk, np.float64)
    has_bias = not np.allclose(bhf, 0.0)
    shared = {
        "wh": f(Whf),
        "wqk": f(Wqkf),
        "bqk": f(bqkf).reshape(QK, 1),
        "gam0": f(gamma[0] / float(S)).reshape(QK, 1),
        "bet0": f(beta[0] / float(S)).reshape(QK, 1),
        "gam1": f(gamma[1]).reshape(QK, 1),
        "bet1": f(beta[1]).reshape(QK, 1),
        "wo": f(Wo).reshape(8, 128, OUT).transpose(1, 0, 2),
        "bo": f(bo).reshape(1, OUT),
    }
    if has_bias:
        shared["bh"] = f(bhf).reshape(1, H2)
    shared = {k: np.ascontiguousarray(v) for k, v in shared.items()}
    in_maps = []
    for c in range(NCORES):
        b, h = c // 2, c % 2
        m = dict(shared)
        m["xkv"] = np.ascontiguousarray(
            np.concatenate(
                [x[b, h * SH : (h + 1) * SH], x[b, (1 - h) * SH : (2 - h) * SH]],
                axis=0,
            )
        )
        in_maps.append(m)
    return has_bias, in_maps


def kernel(x, ln_g, ln_b, Wh, bh, Wqk, bqk, gamma, beta, Wo, bo):
    from concourse.bass_utils import run_bass_kernel_spmd

    has_bias, in_maps = _prep(
        x, ln_g, ln_b, Wh, bh, Wqk, bqk, gamma, beta, Wo, bo
    )
    nc = _get_nc(has_bias=has_bias)
    res = run_bass_kernel_spmd(nc, in_maps, core_ids=list(range(NCORES)))
    out = np.empty((4, S, OUT), dtype=np.float32)
    for c in range(NCORES):
        b, h = c // 2, c % 2
        out[b, h * SH : (h + 1) * SH] = res.results[c]["out"]
    return out


# revision 11
# speedup vs baseline: 1.6286x; 1.6286x over previous
"""GAU (gated attention unit) Bass kernel for Trainium2, 8 NeuronCores.

Sharding: 8 cores = 4 batches x 2 sequence halves. Each core receives its
batch's x with ROWS REORDERED so its own q half comes first; it computes
k/v for all 4096 rows and attention output for rows 0..2047 (its q half).
Row order of kv is irrelevant to attention (sum over j).

Per-core pipeline (v2):
  phase A (kv loop, 8 groups of 512 rows): load x once, LN stats on DVE
    with one batched ACT Sqrt per group, normalize on DVE (f32r out),
    PE-transpose (f32r identity), Z projection -> kt (and qt for groups
    0-3), v projection -> bf16 SBUF, gate projection (groups 0-3) -> bf16
    SBUF (no HBM spill). ACT only runs Silu/Sqrt; Relu/Silu/Sqrt table
    swaps minimized.
  phase C (4 i-blocks of 512 q rows): simT = kT.T @ qT f32r, A=relu^2 ->
    bf16 (ACT relu + DVE square), V = A.T @ v bf16 accumulating in PSUM,
    V *= gate (bf16), PE-transpose bf16, out = Vg @ Wo bf16, + bo, DMA.

Bias matmuls (ones-row trick) are only emitted when the folded bias is
nonzero (compile-time specialization keyed on host-side values).
"""
import os
import sys

sys.path.insert(0, "/opt/trn_rl_repo")

import numpy as np

# CoreSim doesn't implement the Silu activation; silu(x) == x*sigmoid(x)
# exactly, so under GAU_SIM_COMPAT=1 we emit Sigmoid + multiply instead
# (numerically identical, only used for simulator validation).
_SIM_COMPAT = bool(os.environ.get("GAU_SIM_COMPAT"))

import concourse.bass as bass
import concourse.mybir as mybir
from concourse import bacc
from concourse.masks import make_identity
from concourse.tile import TileContext

F32 = mybir.dt.float32
F32R = mybir.dt.float32r
BF16 = mybir.dt.bfloat16
AF = mybir.ActivationFunctionType
OP = mybir.AluOpType

S = 4096          # full sequence (kv rows per core)
SH = 2048         # per-core q rows
D = 512           # model dim
HID = 1024        # v / gate width
H2 = 2048         # 2*HID
QK = 128
OUT = 8
NKV = S // 128    # 32 kv seq tiles
NQ = SH // 128    # 16 q seq tiles
NG = S // 512     # 8 groups of 4 tiles
NGQ = SH // 512   # 4 q groups
NCORES = 8
LN_EPS = 1e-5

_nc_cache = {}


def _build(nreps=1, has_bias=False):
    nc = bacc.Bacc()

    xkv = nc.dram_tensor("xkv", [S, D], F32, kind="ExternalInput")
    wh = nc.dram_tensor("wh", [D, H2], F32, kind="ExternalInput")
    wqk = nc.dram_tensor("wqk", [D, QK], F32, kind="ExternalInput")
    bqk = nc.dram_tensor("bqk", [QK, 1], F32, kind="ExternalInput")
    gam0 = nc.dram_tensor("gam0", [QK, 1], F32, kind="ExternalInput")
    bet0 = nc.dram_tensor("bet0", [QK, 1], F32, kind="ExternalInput")
    gam1 = nc.dram_tensor("gam1", [QK, 1], F32, kind="ExternalInput")
    bet1 = nc.dram_tensor("bet1", [QK, 1], F32, kind="ExternalInput")
    wo = nc.dram_tensor("wo", [128, 8, OUT], F32, kind="ExternalInput")
    bo = nc.dram_tensor("bo", [1, OUT], F32, kind="ExternalInput")
    bh = (
        nc.dram_tensor("bh", [1, H2], F32, kind="ExternalInput")
        if has_bias
        else None
    )
    out_d = nc.dram_tensor("out", [SH, OUT], F32, kind="ExternalOutput")

    with TileContext(nc) as tc:
        with (
            tc.tile_pool(name="persist", bufs=1) as pers,
            tc.tile_pool(name="vpool", bufs=1) as vpool,
        ):
            # ---- persistent constants ----
            ident_f32 = pers.tile([128, 128], F32, tag="identf")
            make_identity(nc, ident_f32)
            ident_fr = pers.tile([128, 128], F32R, tag="identr")
            nc.vector.tensor_copy(out=ident_fr, in_=ident_f32)
            ident_bf = pers.tile([128, 128], BF16, tag="identb")
            nc.vector.tensor_copy(out=ident_bf, in_=ident_f32)

            bqk_col = pers.tile([128, 1], F32, tag="bqk")
            nc.sync.dma_start(out=bqk_col, in_=bqk[:])
            gam0_c = pers.tile([128, 1], F32, tag="g0")
            nc.sync.dma_start(out=gam0_c, in_=gam0[:])
            bet0_c = pers.tile([128, 1], F32, tag="be0")
            nc.sync.dma_start(out=bet0_c, in_=bet0[:])
            gam1_c = pers.tile([128, 1], F32, tag="g1")
            nc.sync.dma_start(out=gam1_c, in_=gam1[:])
            bet1_c = pers.tile([128, 1], F32, tag="be1")
            nc.sync.dma_start(out=bet1_c, in_=bet1[:])
            wo_stage = pers.tile([128, 8, OUT], F32, tag="wos")
            nc.sync.dma_start(out=wo_stage, in_=wo[:])
            wo_sb = pers.tile([128, 8, OUT], BF16, tag="wo")
            nc.vector.tensor_copy(out=wo_sb, in_=wo_stage)
            bo_bc = pers.tile([128, OUT], F32, tag="bo")
            nc.sync.dma_start(out=bo_bc, in_=bo[:].to_broadcast([128, OUT]))

            ones_row = None
            if has_bias:
                ones_f32 = pers.tile([1, 128], F32, tag="ones32")
                nc.vector.memset(ones_f32, 1.0)
                ones_row = pers.tile([1, 128], F32R, tag="ones")
                nc.vector.tensor_copy(out=ones_row, in_=ones_f32)

            # ---- persistent activations ----
            v_sb = vpool.tile([128, NKV, HID], BF16, tag="v")
            gate_sb = vpool.tile([128, NQ, HID], BF16, tag="gate")
            kt_sb = pers.tile([128, S], F32R, tag="kt")
            qt_sb = pers.tile([128, SH], F32R, tag="qt")

            import contextlib

            rep_ctx = (
                tc.For_i(0, nreps, 1) if nreps > 1 else contextlib.nullcontext()
            )
            with rep_ctx:
                _emit_body(
                    nc, tc,
                    dict(xkv=xkv, wh=wh, wqk=wqk, bh=bh, out_d=out_d),
                    dict(ident_fr=ident_fr, ident_bf=ident_bf,
                         bqk_col=bqk_col, gam0_c=gam0_c, bet0_c=bet0_c,
                         gam1_c=gam1_c, bet1_c=bet1_c, wo_sb=wo_sb,
                         bo_bc=bo_bc, ones_row=ones_row,
                         v_sb=v_sb, gate_sb=gate_sb, kt_sb=kt_sb, qt_sb=qt_sb),
                    has_bias,
                )

    nc.compile()
    return nc


def _silu(nc, pool, out, in_, bias=0.0):
    """out = silu(in_ + bias). Real Silu on HW; Sigmoid+mul under sim."""
    if not _SIM_COMPAT:
        nc.scalar.activation(out=out, in_=in_, func=AF.Silu, bias=bias)
        return
    shape = [in_.shape[0], int(np.prod(in_.shape[1:]))]
    sg = pool.tile(shape, F32, tag="silu_sg")
    nc.scalar.activation(out=sg, in_=in_, func=AF.Sigmoid, bias=bias)
    if isinstance(bias, float) and bias == 0.0:
        nc.vector.tensor_mul(out=out, in0=sg, in1=in_)
    else:
        xb = pool.tile(shape, F32, tag="silu_xb")
        nc.vector.tensor_scalar(xb, in_, bias, None, OP.add)
        nc.vector.tensor_mul(out=out, in0=sg, in1=xb)


def _emit_body(nc, tc, drams, sbufs, has_bias):
    xkv, wh, wqk, bh, out_d = (
        drams[k] for k in ("xkv", "wh", "wqk", "bh", "out_d")
    )
    ident_fr = sbufs["ident_fr"]
    ident_bf = sbufs["ident_bf"]
    bqk_col = sbufs["bqk_col"]
    gam0_c, bet0_c = sbufs["gam0_c"], sbufs["bet0_c"]
    gam1_c, bet1_c = sbufs["gam1_c"], sbufs["bet1_c"]
    wo_sb, bo_bc = sbufs["wo_sb"], sbufs["bo_bc"]
    ones_row = sbufs["ones_row"]
    v_sb, gate_sb = sbufs["v_sb"], sbufs["gate_sb"]
    kt_sb, qt_sb = sbufs["kt_sb"], sbufs["qt_sb"]

    # ================= phase A: LN + projections =================
    with (
        tc.tile_pool(name="wp", bufs=1) as wp,
        tc.tile_pool(name="xp", bufs=4) as xp,
        tc.tile_pool(name="sp", bufs=12) as sp,
        tc.tile_pool(name="nscp", bufs=3) as nscp,
        tc.tile_pool(name="nTp", bufs=2) as nTp,
        tc.tile_pool(name="zp", bufs=2) as zp,
        tc.tile_pool(name="slp", bufs=2) as slp,
        tc.tile_pool(name="psTr", bufs=2, space="PSUM") as psTr,
        tc.tile_pool(name="psZ", bufs=2, space="PSUM") as psZ,
        tc.tile_pool(name="psP", bufs=2, space="PSUM") as psP,
    ):
        # stage weights: wh, wqk -> f32r
        whr = wp.tile([128, 4, H2], F32R, tag="whr")
        for c in range(4):
            wh_stage = wp.tile([128, H2], F32, tag="whs")
            nc.sync.dma_start(out=wh_stage, in_=wh[c * 128 : (c + 1) * 128, :])
            nc.vector.tensor_copy(out=whr[:, c, :], in_=wh_stage)
        wqkr = wp.tile([128, 4, QK], F32R, tag="wqkr")
        for c in range(4):
            wqk_stage = wp.tile([128, QK], F32, tag="wqs")
            nc.sync.dma_start(out=wqk_stage, in_=wqk[c * 128 : (c + 1) * 128, :])
            nc.vector.tensor_copy(out=wqkr[:, c, :], in_=wqk_stage)
        bh_row = None
        if has_bias:
            bh_stage = wp.tile([1, H2], F32, tag="bhs")
            nc.sync.dma_start(out=bh_stage, in_=bh[:])
            bh_row = wp.tile([1, H2], F32R, tag="bhr")
            nc.vector.tensor_copy(out=bh_row, in_=bh_stage)

        for g in range(NG):
            is_q = g < NGQ
            # -- LN: stats for 4 tiles, one batched sqrt --
            xts = []
            mvs = []
            varg = sp.tile([128, 4], F32, tag="varg")
            for t in range(4):
                xt = xp.tile([128, D], F32, tag="xt")
                nc.sync.dma_start(
                    out=xt,
                    in_=xkv[(g * 4 + t) * 128 : (g * 4 + t + 1) * 128, :],
                )
                xts.append(xt)
                st = sp.tile([128, 6], F32, tag="bnst")
                nc.vector.bn_stats(out=st, in_=xt)
                mv = sp.tile([128, 2], F32, tag="bnmv")
                nc.vector.bn_aggr(out=mv, in_=st)
                mvs.append(mv)
                nc.vector.tensor_scalar(
                    varg[:, t : t + 1], mv[:, 1:2], LN_EPS, None, OP.add
                )
            sqg = sp.tile([128, 4], F32, tag="sqg")
            nc.scalar.activation(out=sqg, in_=varg, func=AF.Sqrt)
            rstdg = sp.tile([128, 4], F32, tag="rstdg")
            nc.vector.reciprocal(out=rstdg, in_=sqg)
            nmrg = sp.tile([128, 4], F32, tag="nmrg")
            for t in range(4):
                nc.vector.tensor_scalar(
                    nmrg[:, t : t + 1], mvs[t][:, 0:1],
                    rstdg[:, t : t + 1], -1.0, OP.mult, OP.mult,
                )
            # -- normalize (f32r) + transpose --
            nT = nTp.tile([128, 4, 512], F32R, tag="nT")
            for t in range(4):
                nsc = nscp.tile([128, D], F32R, tag="nsc")
                nc.vector.tensor_scalar(
                    nsc, xts[t], rstdg[:, t : t + 1], nmrg[:, t : t + 1],
                    OP.mult, OP.add,
                )
                ptr = psTr.tile([128, 4, 128], F32R, tag="ptr")
                for c in range(4):
                    nc.tensor.transpose(
                        ptr[:, c, :], nsc[:, c * 128 : (c + 1) * 128], ident_fr
                    )
                nc.vector.tensor_copy(
                    out=nT[:, :, t * 128 : (t + 1) * 128], in_=ptr
                )
            # -- Z projection -> kt (and qt) --
            psz = psZ.tile([128, 512], F32, tag="psz")
            for c in range(4):
                nc.tensor.matmul(
                    psz, wqkr[:, c, :], nT[:, c, :],
                    start=(c == 0), stop=(c == 3),
                )
            zs = zp.tile([128, 512], F32, tag="zs")
            _silu(nc, slp, zs, psz, bias=bqk_col)
            nc.vector.tensor_scalar(
                kt_sb[:, g * 512 : (g + 1) * 512], zs,
                gam1_c, bet1_c, OP.mult, OP.add,
            )
            if is_q:
                nc.vector.tensor_scalar(
                    qt_sb[:, g * 512 : (g + 1) * 512], zs,
                    gam0_c, bet0_c, OP.mult, OP.add,
                )
            # -- v projection (and gate for q groups) --
            for t in range(4):
                s_idx = g * 4 + t
                psp = psP.tile([128, HID], F32, tag="psp")
                for nh in range(2):
                    for c in range(4):
                        nc.tensor.matmul(
                            psp[:, nh * 512 : (nh + 1) * 512],
                            nT[:, c, t * 128 : (t + 1) * 128],
                            whr[:, c, nh * 512 : (nh + 1) * 512],
                            start=(c == 0), stop=(c == 3 and not has_bias),
                        )
                    if has_bias:
                        nc.tensor.matmul(
                            psp[:, nh * 512 : (nh + 1) * 512],
                            ones_row,
                            bh_row[0:1, nh * 512 : (nh + 1) * 512],
                            start=False, stop=True,
                        )
                _silu(nc, slp, v_sb[:, s_idx, :], psp)
                if is_q:
                    psg = psP.tile([128, HID], F32, tag="psp")
                    for nh in range(2):
                        for c in range(4):
                            nc.tensor.matmul(
                                psg[:, nh * 512 : (nh + 1) * 512],
                                nT[:, c, t * 128 : (t + 1) * 128],
                                whr[:, c, HID + nh * 512 : HID + (nh + 1) * 512],
                                start=(c == 0), stop=(c == 3 and not has_bias),
                            )
                        if has_bias:
                            nc.tensor.matmul(
                                psg[:, nh * 512 : (nh + 1) * 512],
                                ones_row,
                                bh_row[0:1, HID + nh * 512 : HID + (nh + 1) * 512],
                                start=False, stop=True,
                            )
                    _silu(nc, slp, gate_sb[:, s_idx, :], psg)

    # ================= phase C: attention =================
    with (
        tc.tile_pool(name="atp", bufs=2) as atp,
        tc.tile_pool(name="rtp", bufs=3) as rtp,
        tc.tile_pool(name="vgp", bufs=2) as vgp,
        tc.tile_pool(name="vgtp", bufs=2) as vgtp,
        tc.tile_pool(name="osp", bufs=2) as osp,
        tc.tile_pool(name="psSim", bufs=2, space="PSUM") as psSim,
        tc.tile_pool(name="psV", bufs=2, space="PSUM") as psV,
        tc.tile_pool(name="psT", bufs=1, space="PSUM") as psT,
        tc.tile_pool(name="psO", bufs=1, space="PSUM") as psO,
    ):
        for ib in range(SH // 512):
            at_h = []
            for half in range(2):
                ath = atp.tile([128, 16, 512], BF16, tag="ath")
                at_h.append(ath)
                for j in range(16):
                    jt = half * 16 + j
                    pss = psSim.tile([128, 512], F32, tag="pss")
                    nc.tensor.matmul(
                        pss,
                        kt_sb[:, jt * 128 : (jt + 1) * 128],
                        qt_sb[:, ib * 512 : (ib + 1) * 512],
                        start=True, stop=True,
                    )
                    rt = rtp.tile([128, 512], F32, tag="rt")
                    nc.scalar.activation(out=rt, in_=pss, func=AF.Relu)
                    nc.vector.tensor_mul(out=ath[:, j, :], in0=rt, in1=rt)
            for t in range(4):
                i_idx = ib * 4 + t
                psv = psV.tile([128, HID], F32, tag="psv")
                for jt in range(NKV):
                    a_sl = at_h[jt // 16][:, jt % 16, t * 128 : (t + 1) * 128]
                    for nh in range(2):
                        nc.tensor.matmul(
                            psv[:, nh * 512 : (nh + 1) * 512],
                            a_sl,
                            v_sb[:, jt, nh * 512 : (nh + 1) * 512],
                            start=(jt == 0), stop=(jt == NKV - 1),
                        )
                vg = vgp.tile([128, HID], BF16, tag="vg")
                nc.vector.tensor_mul(out=vg, in0=psv, in1=gate_sb[:, i_idx, :])
                pst = psT.tile([128, 8, 128], BF16, tag="pst")
                for hc in range(8):
                    nc.tensor.transpose(
                        pst[:, hc, :], vg[:, hc * 128 : (hc + 1) * 128], ident_bf
                    )
                vgt = vgtp.tile([128, 8, 128], BF16, tag="vgt")
                nc.vector.tensor_copy(out=vgt, in_=pst)
                pso = psO.tile([128, OUT], F32, tag="pso")
                for hc in range(8):
                    nc.tensor.matmul(
                        pso, vgt[:, hc, :], wo_sb[:, hc, :],
                        start=(hc == 0), stop=(hc == 7),
                    )
                osb = osp.tile([128, OUT], F32, tag="osb")
                nc.vector.tensor_add(out=osb, in0=pso, in1=bo_bc)
                nc.sync.dma_start(
                    out=out_d[i_idx * 128 : (i_idx + 1) * 128, :], in_=osb
                )


def _get_nc(nreps=1, has_bias=False):
    key = (nreps, has_bias)
    if key not in _nc_cache:
        _nc_cache[key] = _build(nreps, has_bias)
    return _nc_cache[key]


def _prep_in_maps(inputs):
    return _prep(**inputs)[1]


def _prep(x, ln_g, ln_b, Wh, bh, Wqk, bqk, gamma, beta, Wo, bo):
    x = np.asarray(x, dtype=np.float32)
    f = lambda a: np.ascontiguousarray(np.asarray(a, dtype=np.float32))
    ln_g = np.asarray(ln_g, np.float64)
    ln_b = np.asarray(ln_b, np.float64)
    Whf = np.asarray(Wh, np.float64) * ln_g[:, None]
    bhf = np.asarray(bh, np.float64) + ln_b @ np.asarray(Wh, np.float64)
    Wqkf = np.asarray(Wqk, np.float64) * ln_g[:, None]
    bqkf = np.asarray(bqk, np.float64) + ln_b @ np.asarray(Wqk, np.float64)
    has_bias = not np.allclose(bhf, 0.0)
    shared = {
        "wh": f(Whf),
        "wqk": f(Wqkf),
        "bqk": f(bqkf).reshape(QK, 1),
        "gam0": f(gamma[0] / float(S)).reshape(QK, 1),
        "bet0": f(beta[0] / float(S)).reshape(QK, 1),
        "gam1": f(gamma[1]).reshape(QK, 1),
        "bet1": f(beta[1]).reshape(QK, 1),
        "wo": f(Wo).reshape(8, 128, OUT).transpose(1, 0, 2),
        "bo": f(bo).reshape(1, OUT),
    }
    if has_bias:
        shared["bh"] = f(bhf).reshape(1, H2)
    shared = {k: np.ascontiguousarray(v) for k, v in shared.items()}
    in_maps = []
    for c in range(NCORES):
        b, h = c // 2, c % 2
        m = dict(shared)
        m["xkv"] = np.ascontiguousarray(
            np.concatenate(
                [x[b, h * SH : (h + 1) * SH], x[b, (1 - h) * SH : (2 - h) * SH]],
                axis=0,
            )
        )
        in_maps.append(m)
    return has_bias, in_maps


def kernel(x, ln_g, ln_b, Wh, bh, Wqk, bqk, gamma, beta, Wo, bo):
    from concourse.bass_utils import run_bass_kernel_spmd

    has_bias, in_maps = _prep(
        x, ln_g, ln_b, Wh, bh, Wqk, bqk, gamma, beta, Wo, bo
    )
    nc = _get_nc(has_bias=has_bias)
    res = run_bass_kernel_spmd(nc, in_maps, core_ids=list(range(NCORES)))
    out = np.empty((4, S, OUT), dtype=np.float32)
    for c in range(NCORES):
        b, h = c // 2, c % 2
        out[b, h * SH : (h + 1) * SH] = res.results[c]["out"]
    return out
